# revision 1
# baseline (speedup 1.0000x reference)
"""TRN2 Bass kernel for nn_Actor (retrieval_knn).

Data-parallel over batch across 8 NeuronCores (8192 rows/core).
Per core: ap_gather embedding lookup (feature-major), fp32 MLP layer-1 on
TensorE, then scores vs the 2489-entry table with W2 absorbed into the
table side (scores = h @ (table@W2).T + table@b2) computed as three
bf16-split pairs (h1*G1 + h1*G2 + h2*G1 + c1 + c2) for fp32-grade
precision at bf16 streaming speed; per 128-row tile a DVE max8 +
max_index gives the argmax directly. The scores phase runs under a
hardware For_i loop to keep the stored program small.
"""
import sys
sys.path.insert(0, '/opt/trn_rl_repo')
import numpy as np
import ml_dtypes

B = 65536
NCORES = 8
BC = B // NCORES            # 8192
NW, NPTAB, EMB = 1807, 2490, 10
NPROJ = NPTAB - 1           # 2489
HID = 40
NTILES = BC // 128          # 64
UNROLL = 1

_cache = {}


def _bf16(x):
    return np.asarray(x, np.float32).astype(ml_dtypes.bfloat16)


def _build(L=1):
    from concourse import bacc, mybir, bass
    from concourse.tile import TileContext
    import concourse.mybir as mb
    dt = mybir.dt
    nc = bacc.Bacc("TRN2", target_bir_lowering=False, debug=False, num_devices=NCORES)

    widx = nc.dram_tensor("widx", [128, 64], dt.int16, kind="ExternalInput")
    pidx = nc.dram_tensor("pidx", [128, 64], dt.int16, kind="ExternalInput")
    wtab16 = nc.dram_tensor("wtab16", [16, NW], dt.float32, kind="ExternalInput")
    ptab16 = nc.dram_tensor("ptab16", [16, NPTAB], dt.float32, kind="ExternalInput")
    w1t = nc.dram_tensor("w1t", [20, HID], dt.float32, kind="ExternalInput")
    b1e = nc.dram_tensor("b1e", [HID, 1], dt.float32, kind="ExternalInput")
    tstk = nc.dram_tensor("tstk", [122, NPROJ], dt.bfloat16, kind="ExternalInput")
    out_ext = nc.dram_tensor("out", [128, NTILES * 8], dt.uint32, kind="ExternalOutput")

    NIDX = BC // 8           # 1024 ids per gather group
    with TileContext(nc) as tc:
        with tc.tile_pool(name="const", bufs=1) as cp, \
             tc.tile_pool(name="work", bufs=1) as wp, \
             tc.tile_pool(name="sc", bufs=1, space="PSUM") as scp:
            t_wtab = cp.tile([128, NW], dt.float32)
            t_ptab = cp.tile([128, NPTAB], dt.float32)
            t_widx = cp.tile([128, 64], dt.int16)
            t_pidx = cp.tile([128, 64], dt.int16)
            t_w1t = cp.tile([20, HID], dt.float32)
            t_b1 = cp.tile([HID, 1], dt.float32)
            t_tstk = cp.tile([122, NPROJ], dt.bfloat16)
            nc.sync.dma_start(out=t_wtab[0:16, :], in_=wtab16.ap())
            nc.sync.dma_start(out=t_ptab[0:16, :], in_=ptab16.ap())
            nc.sync.dma_start(out=t_widx, in_=widx.ap())
            nc.sync.dma_start(out=t_pidx, in_=pidx.ap())
            nc.sync.dma_start(out=t_w1t, in_=w1t.ap())
            nc.sync.dma_start(out=t_b1, in_=b1e.ap())
            nc.sync.dma_start(out=t_tstk, in_=tstk.ap())
            # replicate gather tables into all 8 groups (doubling)
            for src, n in ((t_wtab, NW), (t_ptab, NPTAB)):
                nc.sync.dma_start(out=src[16:32, :], in_=src[0:16, :])
                nc.sync.dma_start(out=src[32:64, :], in_=src[0:32, :])
                nc.sync.dma_start(out=src[64:128, :], in_=src[0:64, :])

            wg = wp.tile([128, NIDX], dt.float32)
            pg = wp.tile([128, NIDX], dt.float32)
            x = wp.tile([20, BC], dt.float32)
            hstack = wp.tile([122, BC], dt.bfloat16)
            h_f32 = wp.tile([HID, BC], dt.float32)
            h1f = wp.tile([HID, BC], dt.float32)
            hr = wp.tile([HID, BC], dt.float32)
            onesrow = wp.tile([2, BC], dt.bfloat16)
            outbuf = wp.tile([128, NTILES * 8], dt.uint32)
            wstage = wp.tile([122, 128], dt.bfloat16)
            m8 = wp.tile([128, 8], dt.float32)
            nc.vector.memset(onesrow, 1.0)
            nc.sync.dma_start(out=hstack[120:122, :], in_=onesrow)
            ps = scp.tile([128, NPROJ], dt.float32)

            for _ in range(L):
                nc.gpsimd.ap_gather(out_ap=wg, in_ap=t_wtab, idxs_ap=t_widx,
                                    channels=128, num_elems=NW, d=1, num_idxs=NIDX)
                nc.gpsimd.ap_gather(out_ap=pg, in_ap=t_ptab, idxs_ap=t_pidx,
                                    channels=128, num_elems=NPTAB, d=1, num_idxs=NIDX)
                for g in range(8):
                    nc.sync.dma_start(out=x[0:10, g * NIDX:(g + 1) * NIDX],
                                        in_=wg[16 * g:16 * g + 10, :])
                    nc.sync.dma_start(out=x[10:20, g * NIDX:(g + 1) * NIDX],
                                        in_=pg[16 * g:16 * g + 10, :])
                # MLP layer 1: psum carved from the scores tile, 4 big chunks
                # double-buffered across partition bases 0 and 64
                for ch in range(4):
                    c0 = ch * 2048
                    p0 = 64 * (ch % 2)
                    hm = ps[p0:p0 + HID, 0:2048]
                    for k in range(4):
                        nc.tensor.matmul(hm[:, k * 512:(k + 1) * 512], lhsT=t_w1t,
                                         rhs=x[:, c0 + k * 512:c0 + (k + 1) * 512],
                                         start=True, stop=True)
                    nc.scalar.activation(h_f32[:, c0:c0 + 2048], hm,
                                         mb.ActivationFunctionType.Relu, bias=t_b1)
                nc.vector.tensor_copy(hstack[0:HID, :], h_f32)
                # h splits: h1 = bf16(h) (done), h2 = bf16(h - h1)
                nc.gpsimd.tensor_copy(h1f, hstack[0:HID, :])
                nc.gpsimd.tensor_sub(hr, h_f32, h1f)
                nc.gpsimd.tensor_copy(hstack[64:104, :], hr)
                nc.sync.dma_start(out=hstack[40:64, :], in_=hstack[0:24, :])
                nc.sync.dma_start(out=hstack[104:120, :], in_=hstack[24:40, :])

                # scores + argmax under a HW loop
                with tc.For_i(0, NTILES, 1, staggered_reset=True) as iv:
                    nc.vector.tensor_copy(wstage, hstack[:, bass.ds(iv * 128, 128)])
                    for s0 in range(0, NPROJ, 512):
                        sw = min(512, NPROJ - s0)
                        nc.tensor.matmul(ps[:, s0:s0 + sw], lhsT=wstage,
                                         rhs=t_tstk[:, s0:s0 + sw],
                                         start=True, stop=True)
                    nc.vector.max(out=m8, in_=ps)
                    nc.vector.max_index(out=outbuf[:, bass.ds(iv * 8, 8)],
                                        in_max=m8, in_values=ps)

            nc.sync.dma_start(out=out_ext.ap(), in_=outbuf)
    nc.compile()
    return nc


def _host_prep(inputs):
    worker_ids = np.asarray(inputs["worker_ids"]).astype(np.int64)
    project_ids = np.asarray(inputs["project_ids"]).astype(np.int64)
    worker_emb = np.asarray(inputs["worker_emb"], dtype=np.float32)
    project_emb = np.asarray(inputs["project_emb"], dtype=np.float32)
    W1 = np.asarray(inputs["W1"], dtype=np.float32)
    b1 = np.asarray(inputs["b1"], dtype=np.float32)
    W2 = np.asarray(inputs["W2"], dtype=np.float32)
    b2 = np.asarray(inputs["b2"], dtype=np.float32)

    table = project_emb[1:]
    G = (table @ W2).astype(np.float32)
    c = (table @ b2).astype(np.float32)
    G1 = _bf16(G)
    G2 = _bf16(G - G1.astype(np.float32))
    c1 = _bf16(c)
    c2 = _bf16(c - c1.astype(np.float32))
    tstk = np.zeros((122, NPROJ), dtype=ml_dtypes.bfloat16)
    tstk[0:40] = G1.T
    tstk[40:64] = G2.T[0:24]
    tstk[64:104] = G1.T
    tstk[104:120] = G2.T[24:40]
    tstk[120] = c1
    tstk[121] = c2

    def gtab16(emb, nrow):
        t = np.zeros((16, nrow), dtype=np.float32)
        t[0:EMB] = emb.T
        return t

    def widx_layout(ids_core):
        # [8 groups, 64 slots, 16 parts] -> [8, 16, 64] -> [128, 64]
        return ids_core.astype(np.int16).reshape(8, 64, 16).transpose(0, 2, 1).reshape(128, 64)

    shared = {
        "wtab16": gtab16(worker_emb, NW), "ptab16": gtab16(project_emb, NPTAB),
        "w1t": W1.T.astype(np.float32).copy(),
        "b1e": b1.reshape(HID, 1).astype(np.float32),
        "tstk": tstk,
    }
    in_maps = []
    for core in range(NCORES):
        sl = slice(core * BC, (core + 1) * BC)
        m = dict(shared)
        m["widx"] = widx_layout(worker_ids[sl])
        m["pidx"] = widx_layout(project_ids[sl])
        in_maps.append(m)
    return in_maps


def _decode(results):
    idx = np.zeros((B,), dtype=np.int64)
    for core in range(NCORES):
        o = results[core]["out"]          # [128, 8*NTILES] uint32
        for t in range(NTILES):
            rows = slice(core * BC + t * 128, core * BC + (t + 1) * 128)
            idx[rows] = o[:, 8 * t]
    return (idx + 1).astype(np.int32).reshape(B, 1)


def kernel(**inputs):
    from concourse.bass_utils import run_bass_kernel_spmd
    in_maps = _host_prep(inputs)
    if "nc1" not in _cache:
        _cache["nc1"] = _build(L=1)
    res = run_bass_kernel_spmd(_cache["nc1"], in_maps, core_ids=list(range(NCORES)))
    return _decode(res.results)



# revision 6
# speedup vs baseline: 43.0641x; 43.0641x over previous
"""TRN2 Bass kernel for nn_Actor (retrieval_knn).

Data-parallel over batch across 8 NeuronCores (8192 rows/core).
Per core: ap_gather embedding lookup (feature-major), MLP layer-1 on
TensorE fed straight from the gather groups (no x-assembly DMAs), then
scores vs the 2489-entry table with W2 absorbed into the table side
(scores = h @ (table@W2).T + table@b2).  The scores matmul uses a
3-term fp16 split (h1@G1 + h1@G2 + h2@G1 + c) which is fp32-grade
(0 argmax flips on the reference inputs) at fp16 streaming speed.
Per 128-row tile the PSUM scores are staged to SBUF by the Scalar
engine (freeing PSUM for the next tile) and DVE max8 + max_index give
the argmax.  Everything is fully unrolled - no hardware loops, no
back-edge barriers.
"""
import sys
sys.path.insert(0, '/opt/trn_rl_repo')
import numpy as np
import ml_dtypes

B = 65536
NCORES = 8
BC = B // NCORES            # 8192
NW, NPTAB, EMB = 1807, 2490, 10
NPROJ = NPTAB - 1           # 2489
HID = 40
NTILES = BC // 128          # 64
NIDX = BC // 8              # 1024 ids per gather group

_cache = {}


def _f16(x):
    return np.asarray(x, np.float32).astype(np.float16)


def _build(L=1):
    from concourse import bacc, mybir, bass
    from concourse.tile import TileContext
    import concourse.mybir as mb
    dt = mybir.dt
    nc = bacc.Bacc("TRN2", target_bir_lowering=False, debug=False, num_devices=NCORES)

    widx = nc.dram_tensor("widx", [128, 64], dt.int16, kind="ExternalInput")
    pidx = nc.dram_tensor("pidx", [128, 64], dt.int16, kind="ExternalInput")
    wtab16 = nc.dram_tensor("wtab16", [16, NW], dt.float32, kind="ExternalInput")
    ptab16 = nc.dram_tensor("ptab16", [16, NPTAB], dt.float32, kind="ExternalInput")
    w1t = nc.dram_tensor("w1t", [20, HID], dt.float32, kind="ExternalInput")
    b1e = nc.dram_tensor("b1e", [HID, 1], dt.float32, kind="ExternalInput")
    gstk = nc.dram_tensor("gstk", [122, NPROJ], dt.float16, kind="ExternalInput")
    out_ext = nc.dram_tensor("out", [128, NTILES * 8], dt.uint32, kind="ExternalOutput")

    with TileContext(nc) as tc:
        with tc.tile_pool(name="const", bufs=1) as cp, \
             tc.tile_pool(name="work", bufs=1) as wp, \
             tc.tile_pool(name="scb", bufs=3) as sp, \
             tc.tile_pool(name="m8p", bufs=2) as mp, \
             tc.tile_pool(name="hps", bufs=1, space="PSUM") as hpp, \
             tc.tile_pool(name="scps", bufs=1, space="PSUM") as scp:
            t_wtab = cp.tile([128, NW], dt.float32)
            t_ptab = cp.tile([128, NPTAB], dt.float32)
            t_widx = cp.tile([128, 64], dt.int16)
            t_pidx = cp.tile([128, 64], dt.int16)
            t_w1t = cp.tile([20, HID], dt.float32)
            t_b1 = cp.tile([HID, 1], dt.float32)
            t_gstk = cp.tile([122, NPROJ], dt.float16)
            nc.sync.dma_start(out=t_wtab[0:16, :], in_=wtab16.ap())
            nc.sync.dma_start(out=t_ptab[0:16, :], in_=ptab16.ap())
            nc.sync.dma_start(out=t_widx, in_=widx.ap())
            nc.sync.dma_start(out=t_pidx, in_=pidx.ap())
            nc.sync.dma_start(out=t_w1t, in_=w1t.ap())
            nc.sync.dma_start(out=t_b1, in_=b1e.ap())
            nc.sync.dma_start(out=t_gstk, in_=gstk.ap())
            # replicate gather tables into all 8 groups (doubling)
            for src in (t_wtab, t_ptab):
                nc.sync.dma_start(out=src[16:32, :], in_=src[0:16, :])
                nc.sync.dma_start(out=src[32:64, :], in_=src[0:32, :])
                nc.sync.dma_start(out=src[64:128, :], in_=src[0:64, :])

            wg = wp.tile([128, NIDX], dt.float32)
            pg = wp.tile([128, NIDX], dt.float32)
            x = wp.tile([20, BC], dt.float32)
            # rows 0-39 h1, 40-63 h1[0:24], 64-103 h2, 104-119 h1[24:40],
            # 120-121 ones (compute engines may only write at partition
            # bases 0/32/64/96; DMA fills the rest)
            hstack = wp.tile([122, BC], dt.float16)
            onesrow = wp.tile([2, BC], dt.float16)
            outbuf = wp.tile([128, NTILES * 8], dt.uint32)
            nc.vector.memset(onesrow, 1.0)
            nc.sync.dma_start(out=hstack[120:122, :], in_=onesrow)

            for _ in range(L):
                nc.gpsimd.ap_gather(out_ap=wg, in_ap=t_wtab, idxs_ap=t_widx,
                                    channels=128, num_elems=NW, d=1, num_idxs=NIDX)
                nc.gpsimd.ap_gather(out_ap=pg, in_ap=t_ptab, idxs_ap=t_pidx,
                                    channels=128, num_elems=NPTAB, d=1, num_idxs=NIDX)
                # MLP layer 1, one gather group (1024 rows) at a time:
                # h = relu(W1 @ [we; pe] + b1), written as fp16 h1,
                # duplicate h1, and fp16 residual h2 = h - h1.
                for g in range(8):
                    c0 = g * NIDX
                    nc.sync.dma_start(out=x[0:10, c0:c0 + NIDX],
                                      in_=wg[16 * g:16 * g + 10, :])
                    nc.sync.dma_start(out=x[10:20, c0:c0 + NIDX],
                                      in_=pg[16 * g:16 * g + 10, :])
                    hp = hpp.tile([HID, NIDX], dt.float32)
                    for k in (0, 512):
                        nc.tensor.matmul(hp[:, k:k + 512], lhsT=t_w1t,
                                         rhs=x[:, c0 + k:c0 + k + 512],
                                         start=True, stop=True)
                    nc.scalar.activation(hstack[0:HID, c0:c0 + NIDX], hp,
                                         mb.ActivationFunctionType.Relu, bias=t_b1)
                    nc.vector.tensor_sub(hstack[64:104, c0:c0 + NIDX],
                                         hp, hstack[0:HID, c0:c0 + NIDX])
                    nc.sync.dma_start(out=hstack[40:64, c0:c0 + NIDX],
                                      in_=hstack[0:24, c0:c0 + NIDX])
                    nc.sync.dma_start(out=hstack[104:120, c0:c0 + NIDX],
                                      in_=hstack[24:40, c0:c0 + NIDX])

                # scores + argmax, one 128-row tile at a time
                for t in range(NTILES):
                    sc = scp.tile([128, NPROJ], dt.float32)
                    lhsT = hstack[:, t * 128:(t + 1) * 128]
                    for s0 in range(0, NPROJ, 512):
                        sw = min(512, NPROJ - s0)
                        nc.tensor.matmul(sc[:, s0:s0 + sw], lhsT=lhsT,
                                         rhs=t_gstk[:, s0:s0 + sw],
                                         start=True, stop=True)
                    scb = sp.tile([128, NPROJ], dt.float32)
                    nc.scalar.activation(scb, sc, mb.ActivationFunctionType.Copy)
                    m8 = mp.tile([128, 8], dt.float32)
                    nc.vector.max(out=m8, in_=scb)
                    nc.vector.max_index(out=outbuf[:, t * 8:(t + 1) * 8],
                                        in_max=m8, in_values=scb)

            nc.sync.dma_start(out=out_ext.ap(), in_=outbuf)
    nc.compile()
    return nc


def _host_prep(inputs):
    worker_ids = np.asarray(inputs["worker_ids"]).astype(np.int64)
    project_ids = np.asarray(inputs["project_ids"]).astype(np.int64)
    worker_emb = np.asarray(inputs["worker_emb"], dtype=np.float32)
    project_emb = np.asarray(inputs["project_emb"], dtype=np.float32)
    W1 = np.asarray(inputs["W1"], dtype=np.float32)
    b1 = np.asarray(inputs["b1"], dtype=np.float32)
    W2 = np.asarray(inputs["W2"], dtype=np.float32)
    b2 = np.asarray(inputs["b2"], dtype=np.float32)

    table = project_emb[1:]
    G = (table @ W2).astype(np.float32)
    c = (table @ b2).astype(np.float32)
    G1 = _f16(G)
    G2 = _f16(G - G1.astype(np.float32))
    c1 = _f16(c)
    c2 = _f16(c - c1.astype(np.float32))
    gstk = np.zeros((122, NPROJ), dtype=np.float16)
    gstk[0:40] = G1.T
    gstk[40:64] = G2.T[0:24]
    gstk[64:104] = G1.T
    gstk[104:120] = G2.T[24:40]
    gstk[120] = c1
    gstk[121] = c2

    def gtab16(emb, nrow):
        t = np.zeros((16, nrow), dtype=np.float32)
        t[0:EMB] = emb.T
        return t

    def idx_layout(ids_core):
        # [8 groups, 64 slots, 16 parts] -> [8, 16, 64] -> [128, 64]
        return ids_core.astype(np.int16).reshape(8, 64, 16).transpose(0, 2, 1).reshape(128, 64)

    shared = {
        "wtab16": gtab16(worker_emb, NW), "ptab16": gtab16(project_emb, NPTAB),
        "w1t": W1.T.astype(np.float32).copy(),
        "b1e": b1.reshape(HID, 1).astype(np.float32),
        "gstk": gstk,
    }
    in_maps = []
    for core in range(NCORES):
        sl = slice(core * BC, (core + 1) * BC)
        m = dict(shared)
        m["widx"] = idx_layout(worker_ids[sl])
        m["pidx"] = idx_layout(project_ids[sl])
        in_maps.append(m)
    return in_maps


def _decode(results):
    idx = np.zeros((B,), dtype=np.int64)
    for core in range(NCORES):
        o = results[core]["out"]          # [128, 8*NTILES] uint32
        for t in range(NTILES):
            rows = slice(core * BC + t * 128, core * BC + (t + 1) * 128)
            idx[rows] = o[:, 8 * t]
    return (idx + 1).astype(np.int32).reshape(B, 1)


def kernel(**inputs):
    from concourse.bass_utils import run_bass_kernel_spmd
    in_maps = _host_prep(inputs)
    if "nc1" not in _cache:
        _cache["nc1"] = _build(L=1)
    res = run_bass_kernel_spmd(_cache["nc1"], in_maps, core_ids=list(range(NCORES)))
    return _decode(res.results)
